# revision 1
# baseline (speedup 1.0000x reference)
"""DiT block kernel for Trainium2 (Bass/Tile), 8-core data parallel.

Shapes (hardcoded from the problem spec):
  x: (8, 1024, 1152), t_emb: (8, 1152)
  w_qkv (1152, 3456), w_proj (1152, 1152), w_fc1 (1152, 4608),
  w_fc2 (4608, 1152), w_ada (1152, 6912) + biases.

Strategy: batch-parallel across 8 cores (one batch element each, no
collectives). Inside a core, activations live in "transposed" layout
[D on partitions, tokens on free] so every projection is
out_T = W.T @ x_T with lhsT = W exactly as stored in DRAM.
LayerNorm statistics reduce over the partition (D) axis via ones-vector
matmuls; softmax runs in transposed orientation (keys on partitions,
no max subtraction -- scores are bounded ~+-8 here) with sums collected
through a ones-column appended to V. Matmuls run in float32r (full PE
rate at N>=256) except attention/proj/fc2 which run in bf16.
"""

import os
import threading
from contextlib import ExitStack

import numpy as np

import concourse.bass as bass
import concourse.mybir as mybir
import concourse.tile as tile
from concourse import bacc
from concourse.bass_utils import run_bass_kernel_spmd
from concourse.masks import make_identity

F32 = mybir.dt.float32
F32R = mybir.dt.float32r
BF16 = mybir.dt.bfloat16
AF = mybir.ActivationFunctionType
ALU = mybir.AluOpType

NCORES = 8
D = 1152
NT = 1024          # tokens per core (batch element)
KT = D // 128      # 9 partition-tiles of D
H = 16
HD = 72
HID = 4 * D        # 4608
MQK = (2 * D) // 128   # 18 output tiles for q,k
MH = HID // 128        # 36
EPS = 1e-6
ISC = 1.0 / float(np.sqrt(HD))

# v output column slices aligned to head boundaries (each >=256 for f32r)
V_SLICES = [(0, 432, 0, 6), (432, 864, 6, 12), (864, 1152, 12, 16)]


def _r(ap):
    return ap.bitcast(F32R)


def _head_segs(d0, n):
    """Split logical rows [d0, d0+n) of a [*,128]-tiled stacked tensor into
    (ktile, part0, length, dst_offset) segments within 128-partition tiles."""
    segs = []
    off = 0
    while n > 0:
        kt_i, p0 = divmod(d0, 128)
        ln = min(n, 128 - p0)
        segs.append((kt_i, p0, ln, off))
        d0 += ln
        off += ln
        n -= ln
    return segs


def _build_program():
    nc = bacc.Bacc(
        "TRN2", target_bir_lowering=False, debug=False, enable_asserts=False
    )
    ins = {}
    ins["x"] = nc.dram_tensor("x", [NT, D], F32, kind="ExternalInput").ap()
    ins["t_emb"] = nc.dram_tensor("t_emb", [D], F32, kind="ExternalInput").ap()
    for name, shape in [
        ("w_qkv", [D, 3 * D]), ("b_qkv", [3 * D]),
        ("w_proj", [D, D]), ("b_proj", [D]),
        ("w_fc1", [D, HID]), ("b_fc1", [HID]),
        ("w_fc2", [HID, D]), ("b_fc2", [D]),
        ("w_ada", [D, 6 * D]), ("b_ada", [6 * D]),
    ]:
        ins[name] = nc.dram_tensor(name, shape, F32, kind="ExternalInput").ap()
    out_dram = nc.dram_tensor("out", [NT, D], F32, kind="ExternalOutput").ap()

    with tile.TileContext(nc) as tc:
        _body(tc, ins, out_dram)
    nc.compile()
    return nc


def _ln_stats_and_modulate(tc, nc, src, dst, ada_pp, shift_c, scale_c,
                           ones_col, pst, pln, ps_st):
    """dst[:,k,:] = ((src-mean)*rstd) * ada_pp[:,scale_c,k] + ada_pp[:,shift_c,k]
    (mean/rstd over the partition (D) axis per token; scale_c holds 1+scale;
    dst is bf16). Stats for both 512-token halves are emitted first so the
    PE finishes them early; applies follow per half so downstream matmuls
    on half 0 can start while half 1 is still being modulated."""
    ps_x, ps_q, st = {}, {}, {}
    for n in range(2):
        nsl = slice(n * 512, (n + 1) * 512)
        ps_x[n] = ps_st.tile([1, 512], F32, tag="st", name=f"psx{n}")
        ps_q[n] = ps_st.tile([1, 512], F32, tag="st", name=f"psq{n}")
        for k in range(KT):
            xb = pln.tile([128, 512], BF16, tag="xb", bufs=3, name="xb")
            nc.scalar.copy(xb[:, :], src[:, k, nsl])
            sq_b = pln.tile([128, 512], BF16, tag="sqb", bufs=3, name="sq_b")
            nc.vector.tensor_mul(sq_b[:, :], src[:, k, nsl], src[:, k, nsl])
            nc.tensor.matmul(
                ps_x[n][:, :], ones_col[:, :], xb[:, :],
                start=(k == 0), stop=(k == KT - 1), skip_group_check=True,
            )
            nc.tensor.matmul(
                ps_q[n][:, :], ones_col[:, :], sq_b[:, :],
                start=(k == 0), stop=(k == KT - 1), skip_group_check=True,
            )
    eps_sb = pst.tile([1, 1], F32, tag="eps", bufs=1, name="eps_sb")
    nc.vector.memset(eps_sb[:, :], EPS)
    for n in range(2):
        # rows: 0 = mean, 1 = E[x^2] -> rstd
        st[n] = pst.tile([1, 2, 512], F32, tag="lnst", bufs=2, name=f"st{n}")
        nc.vector.tensor_scalar_mul(st[n][:, 0, :], ps_x[n][:, :], 1.0 / D)
        work = pst.tile([1, 512], F32, tag="lnwork", bufs=2, name="work")
        nc.vector.tensor_mul(work[:, :], st[n][:, 0, :], st[n][:, 0, :])
        nc.vector.scalar_tensor_tensor(
            st[n][:, 1, :], ps_q[n][:, :], 1.0 / D, work[:, :],
            ALU.mult, ALU.subtract,
        )
        nc.scalar.activation(st[n][:, 1, :], st[n][:, 1, :], AF.Sqrt,
                             bias=eps_sb[:, :], scale=1.0)
        nc.vector.reciprocal(st[n][:, 1, :], st[n][:, 1, :])
    for n in range(2):
        nsl = slice(n * 512, (n + 1) * 512)
        meanB = pln.tile([128, 512], F32, tag="meanB", bufs=2, name="meanB")
        rstdB = pln.tile([128, 512], F32, tag="rstdB", bufs=2, name="rstdB")
        nc.gpsimd.partition_broadcast(meanB[:, :], st[n][:, 0, :])
        nc.gpsimd.partition_broadcast(rstdB[:, :], st[n][:, 1, :])
        for k in range(KT):
            tmp = pln.tile([128, 512], F32, tag="lnt", bufs=3, name="tmp")
            nc.vector.tensor_sub(tmp[:, :], src[:, k, nsl], meanB[:, :])
            nc.vector.tensor_mul(tmp[:, :], tmp[:, :], rstdB[:, :])
            nc.scalar.activation(
                dst[:, k, nsl], tmp[:, :], AF.Identity,
                bias=ada_pp[:, shift_c, k:k + 1],
                scale=ada_pp[:, scale_c, k:k + 1],
            )


def _truncate_out(tc, nc, out_dram):
    with tc.tile_pool(name="ptrunc", bufs=1) as p:
        z = p.tile([128, D], F32, name="z")
        nc.vector.memset(z[:, :], 0.0)
        for tt in range(NT // 128):
            nc.sync.dma_start(out_dram[tt * 128:(tt + 1) * 128, :], z[:, :])


def _body(tc, ins, out_dram):
    nc = tc.nc
    phase_limit = float(os.environ.get("BASS_PHASES", "6"))
    ctx = ExitStack()
    with ctx:
        dram = ctx.enter_context(tc.tile_pool(name="dram", bufs=1, space="DRAM"))
        ada_dr = dram.tile([6 * D], F32)
        w2_dr = dram.tile([HID, D], BF16)

        pers = ctx.enter_context(tc.tile_pool(name="pers", bufs=1))
        ident = pers.tile([128, 128], F32)
        make_identity(nc, ident[:, :])
        ones_col = pers.tile([128, 1], BF16)
        nc.vector.memset(ones_col[:, :], 1.0)
        t_pp = pers.tile([128, KT], F32)
        nc.sync.dma_start(t_pp[:, :], ins["t_emb"].rearrange("(k p) -> p k", p=128))
        t_pb = pers.tile([128, KT], BF16)
        nc.scalar.activation(t_pb[:, :], t_pp[:, :], AF.Silu)

        bqk_pp = pers.tile([128, MQK], F32)
        bv_pp = pers.tile([72, H], F32)
        bproj_pp = pers.tile([128, KT], F32)
        bfc1_pp = pers.tile([128, MH], F32)
        bfc2_pp = pers.tile([128, KT], F32)
        bada_pp = pers.tile([128, 6, KT], F32)
        ada_pp = pers.tile([128, 6, KT], F32)

        def emit_bias_loads():
            nc.sync.dma_start(
                bqk_pp[:, :],
                ins["b_qkv"][0:2 * D].rearrange("(m p) -> p m", p=128))
            nc.sync.dma_start(
                bv_pp[:, :],
                ins["b_qkv"][2 * D:3 * D].rearrange("(h p) -> p h", p=72))
            nc.sync.dma_start(
                bproj_pp[:, :], ins["b_proj"].rearrange("(m p) -> p m", p=128))
            nc.sync.dma_start(
                bfc1_pp[:, :], ins["b_fc1"].rearrange("(m p) -> p m", p=128))
            nc.sync.dma_start(
                bfc2_pp[:, :], ins["b_fc2"].rearrange("(m p) -> p m", p=128))
            nc.sync.dma_start(
                bada_pp[:, :, :],
                ins["b_ada"].rearrange("(c k p) -> p c k", k=KT, p=128))
        xT = pers.tile([128, KT, NT], F32)   # becomes x2T after residual 1
        # weight-stream pool spanning all phases: lets the scheduler prefetch
        # the next phase's weights across pool boundaries
        pw_s = ctx.enter_context(tc.tile_pool(name="pw_s", bufs=1))

        # ============ phase 1: ada, x load+transpose, LN1 ====================
        es_mod1 = ExitStack()
        pmod1 = es_mod1.enter_context(tc.tile_pool(name="pmod1", bufs=1))
        mod1T = pmod1.tile([128, KT, NT], BF16, name="mod1T")

        with tc.tile_pool(name="p1w", bufs=1) as p1w, \
             tc.tile_pool(name="pst", bufs=1) as pst, \
             tc.tile_pool(name="pln", bufs=1) as pln:
            with tc.tile_pool(name="ps_pro", bufs=2, space="PSUM") as ps_pro, \
                 tc.tile_pool(name="pxin", bufs=3) as pxin, \
                 tc.tile_pool(name="ps_tr", bufs=2, space="PSUM") as ps_tr:

                def emit_transpose_block(tt):
                    xin = pxin.tile([128, D], F32, tag="xin", name="xin")
                    nc.sync.dma_start(
                        xin[:, :], ins["x"][tt * 128:(tt + 1) * 128, :])
                    for kd in range(KT):
                        pt = ps_tr.tile([128, 128], F32, tag="ptr", name="pt")
                        nc.tensor.transpose(
                            pt[:, :], xin[:, kd * 128:(kd + 1) * 128],
                            ident[:, :],
                        )
                        tsl = slice(tt * 128, (tt + 1) * 128)
                        if kd % 2 == 0:
                            nc.vector.tensor_copy(xT[:, kd, tsl], pt[:, :])
                        else:
                            nc.scalar.copy(xT[:, kd, tsl], pt[:, :])

                def emit_ada_chunk(n):
                    pa = ps_pro.tile([1, 384], F32, tag="psada", name="pa")
                    for k in range(KT):
                        wada_t = p1w.tile([128, 384], F32, tag="wsk", bufs=4,
                                          name="wada_t")
                        nc.sync.dma_start(
                            wada_t[:, :],
                            ins["w_ada"][k * 128:(k + 1) * 128,
                                         n * 384:(n + 1) * 384],
                        )
                        wada_b = p1w.tile([128, 384], BF16, tag="wskb", bufs=4,
                                          name="wada_b")
                        if k % 2 == 0:
                            nc.gpsimd.tensor_copy(wada_b[:, :], wada_t[:, :])
                        else:
                            nc.vector.tensor_copy(wada_b[:, :], wada_t[:, :])
                        nc.tensor.matmul(
                            pa[:, :], t_pb[:, k:k + 1], wada_b[:, :],
                            start=(k == 0), stop=(k == KT - 1),
                        )
                    asb = pst.tile([1, 384], F32, tag="asb", bufs=2, name="asb")
                    nc.vector.tensor_copy(asb[:, :], pa[:, :])
                    nc.sync.dma_start(
                        ada_dr[n * 384:(n + 1) * 384]
                        .rearrange("(a b) -> a b", a=1),
                        asb[0:1, :],
                    )

                # interleave: ada chunk n and transpose block(s) alternate so
                # the PE fills DMA wait time of one with the other
                for i in range(8):
                    emit_transpose_block(i)
                    if i < 6:
                        emit_ada_chunk(i)
                    if i == 0:
                        emit_bias_loads()
                for c in range(2):
                    nc.sync.dma_start(
                        ada_pp[:, c, :],
                        ada_dr[c * D:(c + 1) * D].rearrange("(k p) -> p k", p=128),
                    )
                nc.vector.tensor_add(ada_pp[:, 0:2, :], ada_pp[:, 0:2, :],
                                     bada_pp[:, 0:2, :])
                nc.vector.tensor_scalar_add(ada_pp[:, 1, :], ada_pp[:, 1, :], 1.0)

            if phase_limit > 0.6:
              with tc.tile_pool(name="ps_st", bufs=4, space="PSUM") as ps_st, \
                 tc.tile_pool(name="ps_bc", bufs=2, space="PSUM") as ps_bc:
                _ln_stats_and_modulate(
                    tc, nc, xT, mod1T, ada_pp, 0, 1, ones_col,
                    pst, pln, ps_st,
                )

        if phase_limit <= 1:
            es_mod1.close()
            return _truncate_out(tc, nc, out_dram)

        # ============ phase 2: qkv ==========================================
        es_qkv = ExitStack()
        pqks = es_qkv.enter_context(tc.tile_pool(name="pqks", bufs=1, side="right"))
        qk_st = pqks.tile([128, MQK, NT], BF16, name="qk_st")
        pvaug = es_qkv.enter_context(
            tc.tile_pool(name="pvaug", bufs=1, side="right"))
        # per head: cols 0..72 = v, col 96 = ones (sum row lands on an
        # aligned PSUM partition), cols 72..96 zero padding
        v_aug = pvaug.tile([128, NT // 128, H, 97], BF16, name="v_aug")
        nc.gpsimd.memset(v_aug[:, :, :, HD:97], 0.0)
        nc.gpsimd.memset(v_aug[:, :, :, 96:97], 1.0)

        with tc.tile_pool(name="p2w", bufs=1) as p2w, \
             tc.tile_pool(name="ps_mm", bufs=4, space="PSUM") as ps_mm:
            for mo in range(MQK):
                wqk_t = pw_s.tile([128, KT, 128], F32, tag="ws", bufs=3,
                                  name="wqk_t")
                nc.sync.dma_start(
                    wqk_t[:, :, :],
                    ins["w_qkv"][:, mo * 128:(mo + 1) * 128]
                    .rearrange("(k p) m -> p k m", p=128),
                )
                wqk_b = pw_s.tile([128, KT, 128], BF16, tag="wsb", bufs=3,
                                  name="wqk_b")
                nc.gpsimd.tensor_copy(wqk_b[:, :, :], wqk_t[:, :, :])
                for n in range(2):
                    pm = ps_mm.tile([128, 512], F32, tag="mm", name="pm")
                    for k in range(KT):
                        nc.tensor.matmul(
                            pm[:, :], wqk_b[:, k, :],
                            mod1T[:, k, n * 512:(n + 1) * 512],
                            start=(k == 0), stop=(k == KT - 1),
                        )
                    nc.scalar.activation(
                        qk_st[:, mo, n * 512:(n + 1) * 512], pm[:, :],
                        AF.Identity, bias=bqk_pp[:, mo:mo + 1], scale=1.0,
                    )
            for (c0, c1, h0, h1) in V_SLICES:
                wv_t = p2w.tile([128, KT, 432], F32, tag="wv", bufs=2,
                                name="wv_t")
                nc.sync.dma_start(
                    wv_t[:, :, 0:c1 - c0],
                    ins["w_qkv"][:, 2 * D + c0:2 * D + c1]
                    .rearrange("(k p) m -> p k m", p=128),
                )
                wv_b = p2w.tile([128, KT, 432], BF16, tag="wvb", bufs=2,
                                name="wv_b")
                nc.gpsimd.tensor_copy(wv_b[:, :, 0:c1 - c0], wv_t[:, :, 0:c1 - c0])
                for tt in range(NT // 128):
                    pmv = ps_mm.tile([128, 512], F32, tag="mm", name="pmv")
                    for k in range(KT):
                        nc.tensor.matmul(
                            pmv[:, 0:c1 - c0],
                            mod1T[:, k, tt * 128:(tt + 1) * 128],
                            wv_b[:, k, 0:c1 - c0],
                            start=(k == 0), stop=(k == KT - 1),
                        )
                    for h in range(h0, h1):
                        nc.vector.tensor_copy(
                            v_aug[:, tt, h, 0:HD],
                            pmv[:, h * HD - c0:(h + 1) * HD - c0],
                        )
        es_mod1.close()
        if phase_limit <= 2:
            es_qkv.close()
            return _truncate_out(tc, nc, out_dram)

        # ============ phase 3: attention ====================================
        es_ao = ExitStack()
        pastk = es_ao.enter_context(tc.tile_pool(name="pastk", bufs=1))
        attn_st = pastk.tile([128, KT, NT], BF16, name="attn_st")

        with tc.tile_pool(name="pheads", bufs=2) as pheads, \
             tc.tile_pool(name="pexp", bufs=3) as pexp, \
             tc.tile_pool(name="pattn", bufs=2) as pattn, \
             tc.tile_pool(name="p3w", bufs=1) as p3w, \
             tc.tile_pool(name="ps_s", bufs=3, space="PSUM") as ps_s, \
             tc.tile_pool(name="ps_av", bufs=4, space="PSUM") as ps_av:
            ps_a3 = ps_s  # [1,384] ada tiles share the pool (own tag, 1 buf)

            def emit_late_ada_chunk(n):
                pa = ps_a3.tile([1, 384], F32, tag="psada3", bufs=1,
                                name="pa3")
                for k in range(KT):
                    wada_t = p3w.tile([128, 384], F32, tag="wsk", bufs=4,
                                      name="wada_t3")
                    nc.sync.dma_start(
                        wada_t[:, :],
                        ins["w_ada"][k * 128:(k + 1) * 128,
                                     n * 384:(n + 1) * 384],
                    )
                    wada_b = p3w.tile([128, 384], BF16, tag="wskb", bufs=3,
                                      name="wada_b3")
                    nc.gpsimd.tensor_copy(wada_b[:, :], wada_t[:, :])
                    nc.tensor.matmul(
                        pa[:, :], t_pb[:, k:k + 1], wada_b[:, :],
                        start=(k == 0), stop=(k == KT - 1),
                    )
                asb = p3w.tile([1, 384], F32, tag="asb", bufs=1, name="asb3")
                nc.vector.tensor_copy(asb[:, :], pa[:, :])
                nc.sync.dma_start(
                    ada_dr[n * 384:(n + 1) * 384]
                    .rearrange("(a b) -> a b", a=1),
                    asb[0:1, :],
                )

            def emit_w2_convert(k):
                w2src = p3w.tile([128, D], F32, tag="w2src", bufs=2,
                                 name="w2src")
                nc.sync.dma_start(
                    w2src[:, :], ins["w_fc2"][k * 128:(k + 1) * 128, :]
                )
                w2b = p3w.tile([128, D], BF16, tag="w2b", bufs=2, name="w2b")
                nc.vector.tensor_copy(w2b[:, :], w2src[:, :])
                nc.sync.dma_start(w2_dr[k * 128:(k + 1) * 128, :], w2b[:, :])

            def emit_filler(h):
                # spread late-ada (12 chunks) and w2 conversion (36 blocks)
                # across the 16 head iterations
                if h < 12:
                    emit_late_ada_chunk(6 + h)
                if h == 11:
                    for c in range(2, 6):
                        nc.sync.dma_start(
                            ada_pp[:, c, :],
                            ada_dr[c * D:(c + 1) * D]
                            .rearrange("(k p) -> p k", p=128),
                        )
                    nc.vector.tensor_add(ada_pp[:, 2:6, :], ada_pp[:, 2:6, :],
                                         bada_pp[:, 2:6, :])
                    nc.vector.tensor_scalar_add(ada_pp[:, 4, :],
                                                ada_pp[:, 4, :], 1.0)
                for k2 in range((h * 36) // H, ((h + 1) * 36) // H):
                    emit_w2_convert(k2)

            for h in range(H):
                emit_filler(h)
                q_h = pheads.tile([72, NT], BF16, tag="qh", name="q_h")
                k_h = pheads.tile([72, NT], BF16, tag="kh", name="k_h")
                for (kt_i, p0, ln, off) in _head_segs(h * HD, HD):
                    nc.sync.dma_start(
                        q_h[off:off + ln, :], qk_st[p0:p0 + ln, kt_i, :]
                    )
                for (kt_i, p0, ln, off) in _head_segs(D + h * HD, HD):
                    nc.sync.dma_start(
                        k_h[off:off + ln, :], qk_st[p0:p0 + ln, kt_i, :]
                    )
                attn_f = pattn.tile([72, NT], F32, tag="attnf", bufs=1,
                                    name="attn_f")
                attn_h = pattn.tile([72, NT], BF16, tag="attnh", name="attn_h")
                for n in range(2):
                    nsl = slice(n * 512, (n + 1) * 512)
                    exp_hn = pexp.tile([128, NT // 128, 512], BF16, tag="exp",
                                       bufs=3, name="exp_hn")
                    for kt_i in range(NT // 128):
                        pss = ps_s.tile([128, 512], F32, tag="s", name="pss")
                        nc.tensor.matmul(
                            pss[:, :], k_h[:, kt_i * 128:(kt_i + 1) * 128],
                            q_h[:, nsl], start=True, stop=True,
                        )
                        nc.scalar.activation(
                            exp_hn[:, kt_i, :], pss[:, :], AF.Exp, scale=ISC
                        )
                    pav = ps_av.tile([97, 512], F32, tag="av", name="pav")
                    for kt_i in range(NT // 128):
                        nc.tensor.matmul(
                            pav[:, :], v_aug[:, kt_i, h, :], exp_hn[:, kt_i, :],
                            start=(kt_i == 0), stop=(kt_i == NT // 128 - 1),
                        )
                    recip = pattn.tile([1, 512], F32, tag="recip", bufs=2,
                                       name="recip")
                    nc.vector.reciprocal(recip[:, :], pav[96:97, :])
                    bca = pattn.tile([72, 512], F32, tag="bca", name="bca")
                    nc.gpsimd.partition_broadcast(bca[:, :], recip[:, :])
                    nc.vector.tensor_mul(attn_f[:, nsl], pav[0:72, :], bca[:, :])
                    nc.vector.tensor_scalar_add(
                        attn_h[:, nsl], attn_f[:, nsl], bv_pp[:, h:h + 1]
                    )
                for (kt_i, p0, ln, off) in _head_segs(h * HD, HD):
                    nc.sync.dma_start(
                        attn_st[p0:p0 + ln, kt_i, :], attn_h[off:off + ln, :]
                    )
        es_qkv.close()
        if phase_limit <= 3:
            es_ao.close()
            return _truncate_out(tc, nc, out_dram)

        # ============ phase 4: proj + residual1 + LN2 ========================
        es_mod2 = ExitStack()
        pmod2 = es_mod2.enter_context(
            tc.tile_pool(name="pmod2", bufs=1, side="right"))
        mod2T = pmod2.tile([128, KT, NT], BF16, name="mod2T")

        with tc.tile_pool(name="p4w", bufs=1) as p4w, \
             tc.tile_pool(name="pst4", bufs=1) as pst4, \
             tc.tile_pool(name="pln4", bufs=1) as pln4:
            with tc.tile_pool(name="ps_mm2", bufs=4, space="PSUM") as ps_mm2:
                for mo in range(KT):
                    wp_f = pw_s.tile([128, KT, 128], F32, tag="ws", bufs=3,
                                     name="wp_f")
                    nc.sync.dma_start(
                        wp_f[:, :, :],
                        ins["w_proj"][:, mo * 128:(mo + 1) * 128]
                        .rearrange("(k p) m -> p k m", p=128),
                    )
                    wp_b = pw_s.tile([128, KT, 128], BF16, tag="wsb", bufs=3,
                                     name="wp_b")
                    nc.gpsimd.tensor_copy(wp_b[:, :, :], wp_f[:, :, :])
                    for n in range(2):
                        nsl = slice(n * 512, (n + 1) * 512)
                        pm2 = ps_mm2.tile([128, 512], F32, tag="mm2", name="pm2")
                        for k in range(KT):
                            nc.tensor.matmul(
                                pm2[:, :], wp_b[:, k, :], attn_st[:, k, nsl],
                                start=(k == 0), stop=(k == KT - 1),
                            )
                        t_sb = p4w.tile([128, 512], F32, tag="tsb", bufs=2,
                                        name="t_sb")
                        nc.scalar.activation(
                            t_sb[:, :], pm2[:, :], AF.Identity,
                            bias=bproj_pp[:, mo:mo + 1], scale=1.0,
                        )
                        nc.vector.scalar_tensor_tensor(
                            xT[:, mo, nsl], t_sb[:, :], ada_pp[:, 2, mo:mo + 1],
                            xT[:, mo, nsl], ALU.mult, ALU.add,
                        )


            with tc.tile_pool(name="ps_st2", bufs=4, space="PSUM") as ps_st2, \
                 tc.tile_pool(name="ps_bc2", bufs=2, space="PSUM") as ps_bc2:
                _ln_stats_and_modulate(
                    tc, nc, xT, mod2T, ada_pp, 3, 4, ones_col,
                    pst4, pln4, ps_st2,
                )
        es_ao.close()
        if phase_limit <= 4:
            es_mod2.close()
            return _truncate_out(tc, nc, out_dram)

        # ============ phase 5: FFN ==========================================
        es_o = ExitStack()
        po = es_o.enter_context(tc.tile_pool(name="po", bufs=1))
        o_full = po.tile([128, KT, NT], F32, name="o_full")

        with tc.tile_pool(name="p5w", bufs=1) as p5w, \
             tc.tile_pool(name="ph", bufs=1) as ph, \
             tc.tile_pool(name="p5h", bufs=1) as p5h, \
             tc.tile_pool(name="ps_f1", bufs=2, space="PSUM") as ps_f1, \
             tc.tile_pool(name="ps_f2", bufs=4, space="PSUM") as ps_f2:
            hT_sb = p5h.tile([128, MH, NT], BF16, name="hT_sb")
            for mo in range(MH):
                wf1_t = pw_s.tile([128, KT, 128], F32, tag="ws", bufs=3,
                                  name="wf1_t")
                nc.sync.dma_start(
                    wf1_t[:, :, :],
                    ins["w_fc1"][:, mo * 128:(mo + 1) * 128]
                    .rearrange("(k p) m -> p k m", p=128),
                )
                wf1_b = pw_s.tile([128, KT, 128], BF16, tag="wsb", bufs=3,
                                  name="wf1_b")
                nc.gpsimd.tensor_copy(wf1_b[:, :, :], wf1_t[:, :, :])
                for n in range(2):
                    pf1 = ps_f1.tile([128, 512], F32, tag="f1", name="pf1")
                    for k in range(KT):
                        nc.tensor.matmul(
                            pf1[:, :], wf1_b[:, k, :],
                            mod2T[:, k, n * 512:(n + 1) * 512],
                            start=(k == 0), stop=(k == KT - 1),
                        )
                    nc.scalar.activation(
                        hT_sb[:, mo, n * 512:(n + 1) * 512], pf1[:, :],
                        AF.Gelu_apprx_tanh,
                        bias=bfc1_pp[:, mo:mo + 1], scale=1.0,
                    )
            # fc2 in groups of 2 m-tiles (4 psum banks) so 2 banks remain
            # for output transposes interleaved right behind each group
            for ms in ([0, 1], [2, 3], [4, 5], [6, 7], [8]):
                pms = {}
                for m in ms:
                    for n in range(2):
                        pms[(m, n)] = ps_f2.tile(
                            [128, 512], F32, tag="f2", bufs=4,
                            name=f"f2_{m}_{n}"
                        )
                w = 128 * len(ms)
                for k in range(MH):
                    w2_rd = p5w.tile([128, 384], BF16, tag="w2rd", bufs=8,
                                     name="w2_rd")
                    nc.sync.dma_start(
                        w2_rd[:, 0:w],
                        w2_dr[k * 128:(k + 1) * 128,
                              ms[0] * 128:ms[0] * 128 + w],
                    )
                    for n in range(2):
                        for i, m in enumerate(ms):
                            nc.tensor.matmul(
                                pms[(m, n)][:, :],
                                w2_rd[:, i * 128:(i + 1) * 128],
                                hT_sb[:, k, n * 512:(n + 1) * 512],
                                start=(k == 0), stop=(k == MH - 1),
                                skip_group_check=True,
                            )
                for m in ms:
                    for n in range(2):
                        nsl = slice(n * 512, (n + 1) * 512)
                        t2 = p5w.tile([128, 512], F32, tag="tsb", bufs=3,
                                      name="t2")
                        nc.scalar.activation(
                            t2[:, :], pms[(m, n)][:, :], AF.Identity,
                            bias=bfc2_pp[:, m:m + 1], scale=1.0,
                        )
                        nc.vector.scalar_tensor_tensor(
                            o_full[:, m, nsl], t2[:, :],
                            ada_pp[:, 5, m:m + 1], xT[:, m, nsl],
                            ALU.mult, ALU.add,
                        )
                    for tt in range(NT // 128):
                        pt = ps_f2.tile([128, 128], F32, tag="tro", bufs=2,
                                        name="pt6")
                        nc.tensor.transpose(
                            pt[:, :], o_full[:, m, tt * 128:(tt + 1) * 128],
                            ident[:, :],
                        )
                        ot = ph.tile([128, 128], F32, tag="ot", bufs=4,
                                     name="ot")
                        if tt % 2 == 0:
                            nc.vector.tensor_copy(ot[:, :], pt[:, :])
                        else:
                            nc.scalar.copy(ot[:, :], pt[:, :])
                        nc.sync.dma_start(
                            out_dram[tt * 128:(tt + 1) * 128,
                                     m * 128:(m + 1) * 128],
                            ot[:, :],
                        )
        es_mod2.close()
        es_o.close()


_LOCK = threading.Lock()
_PROG = None


def _get_program():
    global _PROG
    with _LOCK:
        if _PROG is None:
            _PROG = _build_program()
    return _PROG


def _make_in_maps(inputs):
    arrs = {k: np.ascontiguousarray(np.asarray(v, dtype=np.float32))
            for k, v in inputs.items()}
    in_maps = []
    for c in range(NCORES):
        m = {k: v for k, v in arrs.items() if k not in ("x", "t_emb")}
        m["x"] = np.ascontiguousarray(arrs["x"][c])
        m["t_emb"] = np.ascontiguousarray(arrs["t_emb"][c])
        in_maps.append(m)
    return in_maps


def kernel(**inputs):
    nc = _get_program()
    res = run_bass_kernel_spmd(nc, _make_in_maps(inputs), core_ids=list(range(NCORES)))
    return np.stack([r["out"] for r in res.results], axis=0)


def kernel_traced(inputs, **kw):
    """test-harness helper: returns full BassKernelResults with trace."""
    nc = _get_program()
    return run_bass_kernel_spmd(
        nc, _make_in_maps(inputs), core_ids=list(range(NCORES)), trace=True, **kw
    )



# revision 41
# speedup vs baseline: 1.5226x; 1.5226x over previous
"""DiT block kernel for Trainium2 (Bass/Tile), 8-core data parallel.

Shapes (hardcoded from the problem spec):
  x: (8, 1024, 1152), t_emb: (8, 1152)
  w_qkv (1152, 3456), w_proj (1152, 1152), w_fc1 (1152, 4608),
  w_fc2 (4608, 1152), w_ada (1152, 6912) + biases.

Strategy: batch-parallel across 8 cores (one batch element each).
Activations live feature-major [D on partitions, tokens on free].
All large GEMMs run in fp8e4 with DoubleRow perf mode (two 128-row
contraction tiles per instruction); weights are scaled x16 at
conversion and unscaled in the PSUM->SBUF bias-apply.  LayerNorm
statistics use float32r ones-matmuls (full PE rate, no bf16 copies);
modulate is fused into the LN tail as per-partition scalars.
Attention: scores via DoubleRow over the head dim split [36,2],
exp (shifted by -3 to fit fp8e4) on ACT over 2-bank PSUM tiles,
AV via DoubleRow over key-tile pairs with a ones-column for softmax
sums, normalization on DVE.  attn out is stored [72,16,NT] so proj
runs DoubleRow over head pairs with no scatter DMAs.  ada runs as
f32r matvec streaming (no weight conversion at all).
"""

import os
import threading
from contextlib import ExitStack

import numpy as np

import concourse.bass as bass
import concourse.mybir as mybir
import concourse.tile as tile
from concourse import bacc
from concourse.bass_utils import run_bass_kernel_spmd
from concourse.masks import make_identity

F32 = mybir.dt.float32
F32R = mybir.dt.float32r
BF16 = mybir.dt.bfloat16
FP8 = mybir.dt.float8e4
AF = mybir.ActivationFunctionType
ALU = mybir.AluOpType
DR = mybir.MatmulPerfMode.DoubleRow

NCORES = 8
D = 1152
NT = 1024          # tokens per core (batch element)
KT = D // 128      # 9 partition-tiles of D
H = 16
HD = 72
HID = 4 * D        # 4608
MQK = (2 * D) // 128   # 18 output tiles for q,k
MH = HID // 128        # 36
EPS = 1e-6
ISC = 1.0 / float(np.sqrt(HD))
WS = 16.0          # fp8 weight pre-scale
IWS = 1.0 / WS
ESH = 3.0          # exp shift: exp(s-3) keeps fp8e4 in range

# v output column slices aligned to head boundaries
V_SLICES = [(0, 432, 0, 6), (432, 864, 6, 12), (864, 1152, 12, 16)]


def _r(ap):
    return ap.bitcast(F32R)


def _build_program():
    nc = bacc.Bacc(
        "TRN2", target_bir_lowering=False, debug=False, enable_asserts=False,
        num_devices=NCORES,
    )
    ins = {}
    ins["x"] = nc.dram_tensor("x", [NT, D], F32, kind="ExternalInput").ap()
    ins["t_all"] = nc.dram_tensor(
        "t_all", [NCORES, D], F32, kind="ExternalInput").ap()
    ins["w_ada_sh"] = nc.dram_tensor(
        "w_ada_sh", [D, 6 * D // NCORES], F32, kind="ExternalInput").ap()
    for name, shape in [
        ("w_qkv", [D, 3 * D]), ("b_qkv", [3 * D]),
        ("w_proj", [D, D]), ("b_proj", [D]),
        ("w_fc1", [D, HID]), ("b_fc1", [HID]),
        ("w_fc2", [HID, D]), ("b_fc2", [D]),
        ("b_ada", [6 * D]),
    ]:
        ins[name] = nc.dram_tensor(name, shape, F32, kind="ExternalInput").ap()
    out_dram = nc.dram_tensor("out", [NT, D], F32, kind="ExternalOutput").ap()

    with tile.TileContext(nc) as tc:
        _body(tc, ins, out_dram)
    nc.compile()
    return nc


def _conv8(nc, i, out, in_):
    """fp32 -> fp8 weight conversion with x16 pre-scale, rotating engines."""
    e = i % 3
    if e == 0:
        nc.vector.tensor_scalar_mul(out, in_, WS)
    elif e == 1:
        nc.gpsimd.tensor_scalar_mul(out, in_, WS)
    else:
        nc.scalar.mul(out, in_, WS)


def _truncate_out(tc, nc, out_dram):
    with tc.tile_pool(name="ptrunc", bufs=1) as p:
        z = p.tile([128, D], F32, name="z")
        nc.vector.memset(z[:, :], 0.0)
        for tt in range(NT // 128):
            nc.sync.dma_start(out_dram[tt * 128:(tt + 1) * 128, :], z[:, :])


def _ln_modulate(tc, nc, src, dst, ada_pp, shift_c, scale_c, ones_r,
                 pst, pln, ps_st, sq_engine):
    """dst[:,k,:] (fp8) = modulate(LN(src), ada) in feature-major layout.

    Stats: f32r ones-matmuls per 512-token half (PSUM out limit).
    Apply: full-row [128,1024] ops:
      E_k   = mrB*(1+s_k) - sh_k          (DVE tensor_scalar, 2 scalars)
      t1    = src_k * rstdB               (DVE tensor_tensor)
      dst_k = t1*(1+s_k) - E_k            (DVE scalar_tensor_tensor) -> fp8
    """
    ps_x, ps_q = {}, {}
    for n in range(2):
        nsl = slice(n * 512, (n + 1) * 512)
        ps_x[n] = ps_st.tile([1, 512], F32, tag="stx", name=f"psx{n}")
        ps_q[n] = ps_st.tile([1, 512], F32, tag="stq", name=f"psq{n}")
        for k in range(KT):
            sq = pln.tile([128, 512], F32R, tag="sqb", bufs=2, name="sq")
            if sq_engine == "pool":
                nc.gpsimd.tensor_mul(sq[:, :], src[:, k, nsl], src[:, k, nsl])
            else:
                nc.scalar.square(sq[:, :], src[:, k, nsl])
            nc.tensor.matmul(
                ps_x[n][:, :], ones_r[:, :], src[:, k, nsl],
                start=(k == 0), stop=(k == KT - 1), skip_group_check=True,
            )
            nc.tensor.matmul(
                ps_q[n][:, :], ones_r[:, :], sq[:, :],
                start=(k == 0), stop=(k == KT - 1), skip_group_check=True,
            )
    eps_sb = pst.tile([1, 1], F32, tag="eps", bufs=1, name="eps_sb")
    nc.vector.memset(eps_sb[:, :], EPS)
    # st rows: 0 = mean, 1 = rstd, over full 1024 tokens
    st = pst.tile([1, 2, NT], F32, tag="lnst", bufs=1, name="st")
    for n in range(2):
        nsl = slice(n * 512, (n + 1) * 512)
        nc.vector.tensor_scalar_mul(st[:, 0, nsl], ps_x[n][:, :], 1.0 / D)
        work = pst.tile([1, 512], F32, tag="lnwork", bufs=1, name="work")
        nc.vector.tensor_mul(work[:, :], st[:, 0, nsl], st[:, 0, nsl])
        nc.vector.scalar_tensor_tensor(
            st[:, 1, nsl], ps_q[n][:, :], 1.0 / D, work[:, :],
            ALU.mult, ALU.subtract,
        )
        nc.scalar.activation(st[:, 1, nsl], st[:, 1, nsl], AF.Sqrt,
                             bias=eps_sb[:, :], scale=1.0)
        nc.vector.reciprocal(st[:, 1, nsl], st[:, 1, nsl])
    meanB = pln.tile([128, NT], F32, tag="meanB", bufs=1, name="meanB")
    rstdB = pln.tile([128, NT], F32, tag="rstdB", bufs=1, name="rstdB")
    nc.gpsimd.partition_broadcast(meanB[:, :], st[:, 0, :])
    nc.gpsimd.partition_broadcast(rstdB[:, :], st[:, 1, :])
    mrB = pln.tile([128, NT], F32, tag="mrB", bufs=1, name="mrB")
    nc.vector.tensor_mul(mrB[:, :], meanB[:, :], rstdB[:, :])
    for k in range(KT):
        onep = ada_pp[:, scale_c * KT + k: scale_c * KT + k + 1]
        shft = ada_pp[:, shift_c * KT + k: shift_c * KT + k + 1]
        ek = pln.tile([128, NT], F32, tag="ek", bufs=2, name="ek")
        nc.vector.tensor_scalar(ek[:, :], mrB[:, :], onep, shft,
                                ALU.mult, ALU.subtract)
        t1 = pln.tile([128, NT], F32, tag="t1", bufs=2, name="t1")
        nc.vector.tensor_mul(t1[:, :], src[:, k, :], rstdB[:, :])
        nc.vector.scalar_tensor_tensor(
            dst[:, k, :], t1[:, :], onep, ek[:, :], ALU.mult, ALU.subtract,
        )


def _body(tc, ins, out_dram):
    nc = tc.nc
    phase_limit = float(os.environ.get("BASS_PHASES", "6"))
    ctx = ExitStack()
    with ctx:
        dram = ctx.enter_context(tc.tile_pool(name="dram", bufs=1, space="DRAM"))
        ada_in = dram.tile([6 * D], F32)    # my ada columns for all 8 batches
        ada_dr = dram.tile([6 * D], F32)    # full ada row for my batch

        pers = ctx.enter_context(tc.tile_pool(name="pers", bufs=1))
        identr = pers.tile([128, 128], F32R)
        onef = pers.tile([128, 1], F32)
        nc.vector.memset(onef[:, :], 1.0)
        ones_r = pers.tile([128, 1], F32R)
        nc.vector.tensor_copy(ones_r[:, :], onef[:, :])
        onesr_r = ones_r[:, :]
        neg3 = pers.tile([128, 1], F32)
        nc.vector.memset(neg3[:, :], -ESH)
        t_silA = pers.tile([128, KT, NCORES], F32R)

        bqk_pp = pers.tile([128, MQK], F32)
        bproj_pp = pers.tile([128, KT], F32)
        bfc1_pp = pers.tile([128, MH], F32)
        bfc2_pp = pers.tile([128, KT], F32)
        bada_pp = pers.tile([128, 6 * KT], F32)
        ada_pp = pers.tile([128, 6 * KT], F32)

        def emit_bias_loads():
            nc.sync.dma_start(
                bqk_pp[:, :],
                ins["b_qkv"][0:2 * D].rearrange("(m p) -> p m", p=128))
            nc.sync.dma_start(
                bproj_pp[:, :], ins["b_proj"].rearrange("(m p) -> p m", p=128))
            nc.sync.dma_start(
                bfc1_pp[:, :], ins["b_fc1"].rearrange("(m p) -> p m", p=128))
            nc.sync.dma_start(
                bfc2_pp[:, :], ins["b_fc2"].rearrange("(m p) -> p m", p=128))
            nc.sync.dma_start(
                bada_pp[:, :],
                ins["b_ada"].rearrange("(c k p) -> p (c k)", k=KT, p=128))

        xT = pers.tile([128, KT, NT], F32R)  # becomes x2, then out (in place)
        # weight-stream pool spanning phases (prefetch across boundaries)
        pw_s = ctx.enter_context(tc.tile_pool(name="pw_s", bufs=1))
        # fc2 weights, fp8-converted during attention, consumed in phase 6
        pw2sb = ctx.enter_context(
            tc.tile_pool(name="pw2sb", bufs=1, side="right"))
        w2sb = pw2sb.tile([128, MH, D], FP8, name="w2sb")

        # ============ phase 1: ada-early, x load+transpose, LN1 =============
        es_mod1 = ExitStack()
        pmod1 = es_mod1.enter_context(tc.tile_pool(name="pmod1", bufs=1))
        mod1T = pmod1.tile([128, KT, NT], FP8, name="mod1T")

        with tc.tile_pool(name="p1w", bufs=1) as p1w, \
             tc.tile_pool(name="pst", bufs=1) as pst, \
             tc.tile_pool(name="pln", bufs=1) as pln:
            with tc.tile_pool(name="ps_pro", bufs=2, space="PSUM") as ps_pro, \
                 tc.tile_pool(name="pxin", bufs=2) as pxin, \
                 tc.tile_pool(name="ps_tr", bufs=2, space="PSUM") as ps_tr:

                def emit_transpose_block(tt):
                    xin = pxin.tile([128, D], F32R, tag="xin", name="xin")
                    nc.sync.dma_start(
                        xin[:, :],
                        ins["x"][tt * 128:(tt + 1) * 128, :].bitcast(F32R))
                    for kd in range(KT):
                        pt = ps_tr.tile([128, 128], F32, tag="ptr", name="pt")
                        nc.tensor.matmul(
                            _r(pt[:, :]), xin[:, kd * 128:(kd + 1) * 128],
                            identr[:, :], is_transpose=True,
                        )
                        tsl = slice(tt * 128, (tt + 1) * 128)
                        if kd % 2 == 0:
                            nc.vector.tensor_copy(xT[:, kd, tsl], pt[:, :])
                        else:
                            nc.scalar.copy(xT[:, kd, tsl], pt[:, :])

                def emit_ada_front():
                    id32 = p1w.tile([128, 128], F32, tag="id32", bufs=1,
                                    name="id32")
                    make_identity(nc, id32[:, :])
                    nc.vector.tensor_copy(identr[:, :], id32[:, :])
                    t_in = p1w.tile([NCORES, D], F32, tag="tin", bufs=1,
                                    name="t_in")
                    nc.sync.dma_start(t_in[:, :], ins["t_all"][:, :])
                    t_sal = p1w.tile([NCORES, D], F32R, tag="tsal", bufs=1,
                                     name="t_sal")
                    nc.scalar.activation(t_sal[:, :], t_in[:, :], AF.Silu)
                    # silu(t) for all batches -> feature-major [128, KT, 8]
                    for k in range(KT):
                        ptk = ps_tr.tile([128, 128], F32, tag="ptr",
                                         name="ptk")
                        nc.tensor.matmul(
                            _r(ptk[:, 0:NCORES]),
                            t_sal[:, k * 128:(k + 1) * 128],
                            identr[0:NCORES, 0:NCORES], is_transpose=True,
                        )
                        nc.vector.tensor_copy(t_silA[:, k, :],
                                              ptk[:, 0:NCORES])
                    # my ada column-shard for all batches (2 x 432 cols)
                    for c2 in range(2):
                        wash = p1w.tile([128, KT, 432], F32R, tag="wash",
                                        bufs=1, name="wash")
                        nc.sync.dma_start(
                            wash[:, :, :],
                            ins["w_ada_sh"][:, c2 * 432:(c2 + 1) * 432]
                            .rearrange("(k p) m -> p k m", p=128)
                            .bitcast(F32R),
                        )
                        pada = ps_pro.tile([NCORES, 432], F32, tag="psada",
                                           name="pada")
                        for k in range(KT):
                            nc.tensor.matmul(
                                pada[:, :], t_silA[:, k, :], wash[:, k, :],
                                start=(k == 0), stop=(k == KT - 1),
                            )
                        adasb = pst.tile([NCORES, 432], F32, tag="asb",
                                         bufs=2, name="adasb")
                        nc.vector.tensor_copy(adasb[:, :], pada[:, :])
                        nc.sync.dma_start(
                            ada_in[0:6 * D]
                            .rearrange("(b m) -> b m", b=NCORES)
                            [:, c2 * 432:(c2 + 1) * 432],
                            adasb[:, :],
                        )
                    # exchange: piece b of my columns -> core b; receive my
                    # batch's full ada row in global column order
                    nc.gpsimd.collective_compute(
                        "AllToAll", ALU.bypass,
                        [list(range(NCORES))],
                        ins=[ada_in[0:6 * D]], outs=[ada_dr[0:6 * D]],
                    )

                emit_bias_loads()
                emit_ada_front()
                for i in range(8):
                    emit_transpose_block(i)
                for c in range(6):
                    nc.sync.dma_start(
                        ada_pp[:, c * KT:(c + 1) * KT],
                        ada_dr[c * D:(c + 1) * D].rearrange("(k p) -> p k", p=128),
                    )
                nc.vector.tensor_add(ada_pp[:, :], ada_pp[:, :],
                                     bada_pp[:, :])
                nc.vector.tensor_scalar_add(
                    ada_pp[:, KT:2 * KT], ada_pp[:, KT:2 * KT], 1.0)
                nc.vector.tensor_scalar_add(
                    ada_pp[:, 4 * KT:5 * KT], ada_pp[:, 4 * KT:5 * KT], 1.0)

            if phase_limit > 0.6:
                with tc.tile_pool(name="ps_st", bufs=4, space="PSUM") as ps_st:
                    _ln_modulate(
                        tc, nc, xT, mod1T, ada_pp, 0, 1, onesr_r,
                        pst, pln, ps_st, sq_engine="pool",
                    )

        if phase_limit <= 1:
            es_mod1.close()
            return _truncate_out(tc, nc, out_dram)

        # ============ phase 2: qkv =========================================
        es_qkv = ExitStack()
        pqks = es_qkv.enter_context(tc.tile_pool(name="pqks", bufs=1, side="right"))
        qk_st = pqks.tile([128, MQK, NT], FP8, name="qk_st")
        pvaug = es_qkv.enter_context(
            tc.tile_pool(name="pvaug", bufs=1, side="right"))
        # per head: cols 0..72 = v + b_v, col 96 = ones (32-aligned sum row)
        v_aug = pvaug.tile([128, NT // 128, H, 97], FP8, name="v_aug")
        nc.gpsimd.memset(v_aug[:, :, :, HD:96], 0.0)
        nc.gpsimd.memset(v_aug[:, :, :, 96:97], 1.0)

        with tc.tile_pool(name="p2w", bufs=1) as p2w, \
             tc.tile_pool(name="ps_mm", bufs=4, space="PSUM") as ps_mm:
            # bias row for v (broadcast along partitions), built once
            bv_row = p2w.tile([1, D], F32, tag="bvr", bufs=1, name="bv_row")
            nc.sync.dma_start(
                bv_row[:, :],
                ins["b_qkv"][2 * D:3 * D].rearrange("(a b) -> a b", a=1))
            bvB = p2w.tile([128, D], F32, tag="bvB", bufs=1, name="bvB")
            nc.gpsimd.partition_broadcast(bvB[:, :], bv_row[:, :])

            for mo in range(MQK):
                wqk_t = pw_s.tile([128, KT, 128], F32, tag="ws", bufs=3,
                                  name="wqk_t")
                nc.sync.dma_start(
                    wqk_t[:, :, :],
                    ins["w_qkv"][:, mo * 128:(mo + 1) * 128]
                    .rearrange("(k p) m -> p k m", p=128),
                )
                wqk_8 = pw_s.tile([128, KT, 128], FP8, tag="ws8", bufs=3,
                                  name="wqk_8")
                _conv8(nc, mo, wqk_8[:, :, :], wqk_t[:, :, :])
                for n in range(2):
                    nsl = slice(n * 512, (n + 1) * 512)
                    pm = ps_mm.tile([128, 512], F32, tag="mm", name="pm")
                    for i in range(4):
                        nc.tensor.matmul(
                            pm[:, :], wqk_8[:, 2 * i:2 * i + 2, :],
                            mod1T[:, 2 * i:2 * i + 2, nsl],
                            start=(i == 0), stop=False, perf_mode=DR,
                            skip_group_check=True,
                        )
                    nc.tensor.matmul(
                        pm[:, :], wqk_8[:, 8, :], mod1T[:, 8, nsl],
                        start=False, stop=True, skip_group_check=True,
                    )
                    nc.scalar.activation(
                        qk_st[:, mo, nsl], pm[:, :],
                        AF.Identity, bias=bqk_pp[:, mo:mo + 1], scale=IWS,
                    )
            for si, (c0, c1, h0, h1) in enumerate(V_SLICES):
                cw = c1 - c0
                wv_t = p2w.tile([128, KT, 432], F32, tag="wv", bufs=2,
                                name="wv_t")
                nc.sync.dma_start(
                    wv_t[:, :, 0:cw],
                    ins["w_qkv"][:, 2 * D + c0:2 * D + c1]
                    .rearrange("(k p) m -> p k m", p=128),
                )
                wv_8 = p2w.tile([128, KT, 432], FP8, tag="wv8", bufs=2,
                                name="wv_8")
                _conv8(nc, si, wv_8[:, :, 0:cw], wv_t[:, :, 0:cw])
                for tt in range(NT // 128):
                    ttsl = slice(tt * 128, (tt + 1) * 128)
                    pmv = ps_mm.tile([128, 512], F32, tag="mm", name="pmv")
                    for i in range(4):
                        nc.tensor.matmul(
                            pmv[:, 0:cw], mod1T[:, 2 * i:2 * i + 2, ttsl],
                            wv_8[:, 2 * i:2 * i + 2, 0:cw],
                            start=(i == 0), stop=False, perf_mode=DR,
                            skip_group_check=True,
                        )
                    nc.tensor.matmul(
                        pmv[:, 0:cw], mod1T[:, 8, ttsl], wv_8[:, 8, 0:cw],
                        start=False, stop=True, skip_group_check=True,
                    )
                    # v_aug = psum/16 + b_v  (softmax-normalizes to attn+b_v)
                    nc.vector.scalar_tensor_tensor(
                        v_aug[:, tt, h0:h1, 0:HD],
                        pmv[:, 0:cw], IWS, bvB[:, c0:c1],
                        ALU.mult, ALU.add,
                    )
        es_mod1.close()
        if phase_limit <= 2:
            es_qkv.close()
            return _truncate_out(tc, nc, out_dram)

        # ============ phase 3: attention ====================================
        es_ao = ExitStack()
        pastk = es_ao.enter_context(tc.tile_pool(name="pastk", bufs=1))
        attn_st = pastk.tile([72, H, NT], FP8, name="attn_st")

        with tc.tile_pool(name="pheads", bufs=2) as pheads, \
             tc.tile_pool(name="pexp", bufs=3) as pexp, \
             tc.tile_pool(name="pattn", bufs=2) as pattn, \
             tc.tile_pool(name="p3w", bufs=1) as p3w, \
             tc.tile_pool(name="ps_sc", bufs=2, space="PSUM") as ps_sc, \
             tc.tile_pool(name="ps_av", bufs=2, space="PSUM") as ps_av:

            def emit_w2_convert(kp):
                # loads+converts k-tile pair (2*kp, 2*kp+1)
                w2src = p3w.tile([128, 2, D], F32, tag="w2src", bufs=2,
                                 name="w2src")
                nc.scalar.dma_start(
                    w2src[:, :, :],
                    ins["w_fc2"][2 * kp * 128:(2 * kp + 2) * 128, :]
                    .rearrange("(k p) m -> p k m", p=128),
                )
                eng = nc.vector if kp % 2 == 0 else nc.gpsimd
                eng.tensor_scalar_mul(
                    w2sb[:, 2 * kp:2 * kp + 2, :], w2src[:, :, :], WS)

            for h in range(H):
                if h < MH // 2:
                    emit_w2_convert(h)
                if H + h < MH // 2:
                    emit_w2_convert(H + h)
                # gather q,k for head h into [36, 2, NT] (slots = feature
                # pairs; DoubleRow sums slots so any consistent split works)
                q3 = pheads.tile([36, 2, NT], FP8, tag="qh", name="q3")
                k3 = pheads.tile([36, 2, NT], FP8, tag="kh", name="k3")
                for dst, base in ((q3, h * HD), (k3, D + h * HD)):
                    off = 0
                    while off < HD:
                        kt_i, p0 = divmod(base + off, 128)
                        ln = min(HD - off, 128 - p0)
                        nc.sync.dma_start(
                            dst[off // 2:(off + ln) // 2, :, :],
                            qk_st[p0:p0 + ln, kt_i, :],
                        )
                        off += ln
                for n in range(2):
                    nsl = slice(n * 512, (n + 1) * 512)
                    pav = ps_av.tile([97, 512], F32, tag="av", name="pav")
                    for kp in range(4):
                        pss = ps_sc.tile([128, 2, 512], F32, tag="s",
                                         name="pss")
                        for j in range(2):
                            nc.tensor.matmul(
                                pss[:, j, :],
                                k3[:, :, (2 * kp + j) * 128:
                                   (2 * kp + j + 1) * 128],
                                q3[:, :, nsl], start=True, stop=True,
                                perf_mode=DR, skip_group_check=True,
                            )
                        exp_p = pexp.tile([128, 2, 512], FP8, tag="exp",
                                          bufs=3, name="exp_p")
                        nc.scalar.activation(
                            exp_p[:, :, :], pss[:, :, :], AF.Exp,
                            scale=ISC, bias=neg3[:, :],
                        )
                        nc.tensor.matmul(
                            pav[:, :], v_aug[:, 2 * kp:2 * kp + 2, h, :],
                            exp_p[:, :, :],
                            start=(kp == 0), stop=(kp == 3),
                            perf_mode=DR, skip_group_check=True,
                        )
                    recip = pattn.tile([1, 512], F32, tag="recip", bufs=2,
                                       name="recip")
                    nc.vector.reciprocal(recip[:, :], pav[96:97, :])
                    bca = pattn.tile([72, 512], F32, tag="bca", name="bca")
                    nc.gpsimd.partition_broadcast(bca[:, :], recip[:, :])
                    nc.vector.tensor_mul(
                        attn_st[:, h, nsl], pav[0:HD, :], bca[:, :])
        es_qkv.close()
        if phase_limit <= 3:
            es_ao.close()
            return _truncate_out(tc, nc, out_dram)

        # ============ phase 4: proj + residual1 + LN2 =======================
        es_mod2 = ExitStack()
        pmod2 = es_mod2.enter_context(
            tc.tile_pool(name="pmod2", bufs=1, side="right"))
        mod2T = pmod2.tile([128, KT, NT], FP8, name="mod2T")

        with tc.tile_pool(name="p4w", bufs=1) as p4w:
            with tc.tile_pool(name="ps_mm2", bufs=4, space="PSUM") as ps_mm2:
                for mo in range(KT):
                    wp_f = p4w.tile([72, H, 128], F32, tag="wp", bufs=2,
                                    name="wp_f")
                    nc.sync.dma_start(
                        wp_f[:, :, :],
                        ins["w_proj"][:, mo * 128:(mo + 1) * 128]
                        .rearrange("(h p) m -> p h m", p=HD),
                    )
                    wp_8 = p4w.tile([72, H, 128], FP8, tag="wp8", bufs=2,
                                    name="wp_8")
                    _conv8(nc, mo, wp_8[:, :, :], wp_f[:, :, :])
                    for n in range(2):
                        nsl = slice(n * 512, (n + 1) * 512)
                        pm2 = ps_mm2.tile([128, 512], F32, tag="mm2",
                                          name="pm2")
                        for hp in range(H // 2):
                            nc.tensor.matmul(
                                pm2[:, :], wp_8[:, 2 * hp:2 * hp + 2, :],
                                attn_st[:, 2 * hp:2 * hp + 2, nsl],
                                start=(hp == 0), stop=(hp == H // 2 - 1),
                                perf_mode=DR, skip_group_check=True,
                            )
                        t_sb = p4w.tile([128, 512], F32, tag="tsb", bufs=2,
                                        name="t_sb")
                        nc.scalar.activation(
                            t_sb[:, :], pm2[:, :], AF.Identity,
                            bias=bproj_pp[:, mo:mo + 1], scale=IWS,
                        )
                        nc.vector.scalar_tensor_tensor(
                            xT[:, mo, nsl], t_sb[:, :],
                            ada_pp[:, 2 * KT + mo:2 * KT + mo + 1],
                            xT[:, mo, nsl], ALU.mult, ALU.add,
                        )
        es_ao.close()

        with tc.tile_pool(name="pst4", bufs=1) as pst4, \
             tc.tile_pool(name="pln4", bufs=1) as pln4:
            with tc.tile_pool(name="ps_st2", bufs=4, space="PSUM") as ps_st2:
                _ln_modulate(
                    tc, nc, xT, mod2T, ada_pp, 3, 4, onesr_r,
                    pst4, pln4, ps_st2, sq_engine="pool",
                )
        if phase_limit <= 4:
            es_mod2.close()
            return _truncate_out(tc, nc, out_dram)

        # ============ phase 5: fc1 =========================================
        es_h = ExitStack()
        ph5 = es_h.enter_context(tc.tile_pool(name="ph5", bufs=1))
        hT = ph5.tile([128, MH, NT], FP8, name="hT")

        with tc.tile_pool(name="ps_f1", bufs=4, space="PSUM") as ps_f1:
            for mo in range(MH):
                wf1_t = pw_s.tile([128, KT, 128], F32, tag="ws", bufs=3,
                                  name="wf1_t")
                nc.sync.dma_start(
                    wf1_t[:, :, :],
                    ins["w_fc1"][:, mo * 128:(mo + 1) * 128]
                    .rearrange("(k p) m -> p k m", p=128),
                )
                wf1_8 = pw_s.tile([128, KT, 128], FP8, tag="ws8", bufs=3,
                                  name="wf1_8")
                _conv8(nc, mo, wf1_8[:, :, :], wf1_t[:, :, :])
                for n in range(2):
                    nsl = slice(n * 512, (n + 1) * 512)
                    pf1 = ps_f1.tile([128, 512], F32, tag="f1", name="pf1")
                    for i in range(4):
                        nc.tensor.matmul(
                            pf1[:, :], wf1_8[:, 2 * i:2 * i + 2, :],
                            mod2T[:, 2 * i:2 * i + 2, nsl],
                            start=(i == 0), stop=False, perf_mode=DR,
                            skip_group_check=True,
                        )
                    nc.tensor.matmul(
                        pf1[:, :], wf1_8[:, 8, :], mod2T[:, 8, nsl],
                        start=False, stop=True, skip_group_check=True,
                    )
                    nc.scalar.activation(
                        hT[:, mo, nsl], pf1[:, :], AF.Gelu_apprx_tanh,
                        bias=bfc1_pp[:, mo:mo + 1], scale=IWS,
                    )
        es_mod2.close()
        if phase_limit <= 5:
            es_h.close()
            return _truncate_out(tc, nc, out_dram)

        # ============ phase 6: fc2 + residual2 + output =====================
        with tc.tile_pool(name="p6", bufs=1) as p6, \
             tc.tile_pool(name="ps_f2", bufs=6, space="PSUM") as ps_f2, \
             tc.tile_pool(name="ps_tro", bufs=2, space="PSUM") as ps_tro:
            obuf = {}
            for tt in range(NT // 128):
                obuf[tt] = p6.tile([128, KT, 128], F32, tag=f"ob{tt}",
                                   bufs=1, name=f"obuf{tt}")
            for ms in ([0, 1, 2], [3, 4, 5], [6, 7, 8]):
                pms = {}
                for m in ms:
                    for n in range(2):
                        pms[(m, n)] = ps_f2.tile(
                            [128, 512], F32, tag="f2", name=f"f2_{m}_{n}"
                        )
                for k in range(MH // 2):
                    for n in range(2):
                        nsl = slice(n * 512, (n + 1) * 512)
                        for m in ms:
                            nc.tensor.matmul(
                                pms[(m, n)][:, :],
                                w2sb[:, 2 * k:2 * k + 2,
                                     m * 128:(m + 1) * 128],
                                hT[:, 2 * k:2 * k + 2, nsl],
                                start=(k == 0), stop=(k == MH // 2 - 1),
                                perf_mode=DR, skip_group_check=True,
                            )
                for m in ms:
                    for n in range(2):
                        nsl = slice(n * 512, (n + 1) * 512)
                        t2 = p6.tile([128, 512], F32, tag="tsb", bufs=3,
                                     name="t2")
                        nc.scalar.activation(
                            t2[:, :], pms[(m, n)][:, :], AF.Identity,
                            bias=bfc2_pp[:, m:m + 1], scale=IWS,
                        )
                        nc.vector.scalar_tensor_tensor(
                            xT[:, m, nsl], t2[:, :],
                            ada_pp[:, 5 * KT + m:5 * KT + m + 1],
                            xT[:, m, nsl], ALU.mult, ALU.add,
                        )
                    for tt in range(NT // 128):
                        pt = ps_tro.tile([128, 128], F32, tag="tro",
                                         name="pt6")
                        nc.tensor.matmul(
                            _r(pt[:, :]),
                            xT[:, m, tt * 128:(tt + 1) * 128],
                            identr[:, :], is_transpose=True,
                        )
                        if tt % 2 == 0:
                            nc.vector.tensor_copy(obuf[tt][:, m, :], pt[:, :])
                        else:
                            nc.scalar.copy(obuf[tt][:, m, :], pt[:, :])
            for tt in range(NT // 128):
                nc.sync.dma_start(
                    out_dram[tt * 128:(tt + 1) * 128, :],
                    obuf[tt][:, :, :],
                )
        es_h.close()


_LOCK = threading.Lock()
_PROG = None


def _get_program():
    global _PROG
    with _LOCK:
        if _PROG is None:
            _PROG = _build_program()
    return _PROG


def _make_in_maps(inputs):
    arrs = {k: np.ascontiguousarray(np.asarray(v, dtype=np.float32))
            for k, v in inputs.items()}
    in_maps = []
    ash = 6 * D // NCORES
    for c in range(NCORES):
        m = {k: v for k, v in arrs.items()
             if k not in ("x", "t_emb", "w_ada")}
        m["x"] = np.ascontiguousarray(arrs["x"][c])
        m["t_all"] = arrs["t_emb"]
        m["w_ada_sh"] = np.ascontiguousarray(
            arrs["w_ada"][:, c * ash:(c + 1) * ash])
        in_maps.append(m)
    return in_maps


def kernel(**inputs):
    nc = _get_program()
    res = run_bass_kernel_spmd(nc, _make_in_maps(inputs), core_ids=list(range(NCORES)))
    return np.stack([r["out"] for r in res.results], axis=0)


def kernel_traced(inputs, **kw):
    """test-harness helper: returns full BassKernelResults with trace."""
    nc = _get_program()
    return run_bass_kernel_spmd(
        nc, _make_in_maps(inputs), core_ids=list(range(NCORES)), trace=True, **kw
    )
